# revision 17
# baseline (speedup 1.0000x reference)
"""Multi-head causal attention (B=1, S=4096, D=768, H=12) on 8 trn2 NeuronCores.

Sharding: tensor-parallel over heads + causal-balanced split of the query range.
  - cores 0-5 ("late"):  2 heads each, q in [1792, 4096), k in [0, 4096)
  - cores 6-7 ("early"): 6 heads each, q in [0, 1792),  k in [0, 1792)

v3: all-bf16 datapath (fp32 PSUM accumulation); structured to keep the PE's
HAM clock-gate warm: x resident in SBUF (no DMA stalls between matmuls),
projections/transposes interleaved per chunk, k-outer flash attention with
both heads processed per k-tile (one exp per k-tile covering both heads),
out-projection deferred one segment so it fills PE gaps during normalize.
V bias folded into the host-side output bias; bf16 partial outputs summed
on host in fp32.
"""

import os
import sys
import threading

sys.path.insert(0, "/opt/trn_rl_repo")

import numpy as np
import ml_dtypes

import concourse.bass as bass
import concourse.mybir as mybir
import concourse.tile as tile
from concourse import bacc
from concourse.masks import make_identity

# ---------------------------------------------------------------- constants
B, S, D, H, DH = 1, 4096, 768, 12, 64
SCALE = DH ** -0.5
P = 128          # sbuf partitions
KT = 128         # key tile (partition axis of scores)
SEG = 512        # query segment (free axis of scores; one psum bank)
SPLIT = 1536     # early/late query split point
DT = mybir.dt.float32
BF = mybir.dt.bfloat16

CLASSES = {
    # name: (n_pairs, q0, q1, k_len)
    "late": (1, SPLIT, S, S),
    "early": (3, 0, SPLIT, SPLIT),
}


def build_module(cls, debug_taps=False):
    n_pairs, q0, q1, k_len = CLASSES[cls]
    f_c = 128 * n_pairs          # per-core feature width of each projection
    q_len = q1 - q0
    n_kt = k_len // KT           # ktiles of the core's k-support
    n_dt = D // P                # 6 contraction tiles for the projections
    segs = []
    h0 = 0
    while h0 < q_len:
        segs.append((h0, min(h0 + SEG, q_len)))
        h0 += SEG

    nc = bacc.Bacc("TRN2", target_bir_lowering=False, debug=False,
                   enable_asserts=True, num_devices=1)

    xT = nc.dram_tensor("xT", [D, k_len], BF, kind="ExternalInput")
    wqT = nc.dram_tensor("wqT", [D, f_c], BF, kind="ExternalInput")
    wkT = nc.dram_tensor("wkT", [D, f_c], BF, kind="ExternalInput")
    wvT = nc.dram_tensor("wvT", [D, f_c], BF, kind="ExternalInput")
    bq = nc.dram_tensor("bq", [n_pairs * P, 1], DT, kind="ExternalInput")
    woT = nc.dram_tensor("woT", [f_c, D], BF, kind="ExternalInput")
    dmask = nc.dram_tensor("dmask", [P, P], BF, kind="ExternalInput")
    yT = nc.dram_tensor("yT", [D, q_len], BF, kind="ExternalOutput")
    if debug_taps:
        qTd = nc.dram_tensor("qTd", [n_pairs, P, q_len], BF, kind="ExternalOutput")
        kTd = nc.dram_tensor("kTd", [n_pairs, P, k_len], BF, kind="ExternalOutput")
        vktd = nc.dram_tensor("vktd", [n_pairs, P, 132 * n_kt], BF,
                              kind="ExternalOutput")
        aTd = nc.dram_tensor("aTd", [n_pairs, P, q_len], BF, kind="ExternalOutput")

    with tile.TileContext(nc) as tc:
        with (
            tc.tile_pool(name="w", bufs=1) as sb_w,
            tc.tile_pool(name="persist", bufs=1) as sb_per,
            tc.tile_pool(name="vtmp", bufs=3) as sb_vt,
            tc.tile_pool(name="exp", bufs=4) as sb_exp,
            tc.tile_pool(name="rn", bufs=2) as sb_rn,
            tc.tile_pool(name="yout", bufs=2) as sb_y,
        ):
            # ---------------- constants / weights / x to SBUF
            wq_sb = sb_w.tile([P, n_dt, f_c], BF, tag="wq")
            nc.sync.dma_start(out=wq_sb, in_=wqT.rearrange("(t p) f -> p t f", p=P))
            wk_sb = sb_w.tile([P, n_dt, f_c], BF, tag="wk")
            nc.sync.dma_start(out=wk_sb, in_=wkT.rearrange("(t p) f -> p t f", p=P))
            wv_sb = sb_w.tile([P, n_dt, f_c], BF, tag="wv")
            nc.sync.dma_start(out=wv_sb, in_=wvT.rearrange("(t p) f -> p t f", p=P))
            bq_sb = sb_w.tile([P, n_pairs], DT, tag="bq")
            nc.sync.dma_start(out=bq_sb, in_=bq.rearrange("(n p) o -> p (n o)", p=P))
            wo_sb = sb_w.tile([P, n_pairs, n_dt, P], BF, tag="wo")
            nc.sync.dma_start(
                out=wo_sb,
                in_=woT.rearrange("(n p) (t m) -> p n t m", p=P, m=P))
            dmask_sb = sb_w.tile([P, P], BF, tag="dmask")
            nc.sync.dma_start(out=dmask_sb, in_=dmask.ap())
            ident = sb_w.tile([P, P], BF, tag="ident")
            make_identity(nc, ident)

            # whole xT resident in SBUF; chunk DMAs all issued upfront
            x_sb = sb_per.tile([P, n_dt, k_len], BF, name="x_sb", tag="x_sb")
            chunks = []
            s0 = 0
            while s0 < k_len:
                w = min(512, k_len - s0)
                chunks.append((s0, w))
                s0 += w
            for (s0, w) in chunks:
                nc.sync.dma_start(
                    out=x_sb[:, :, s0:s0 + w],
                    in_=xT.rearrange("(t p) s -> p t s", p=P)[:, :, s0:s0 + w])

            # ---------------- persistent activations (head pair packed on
            # partitions: head A rows 0-63, head B rows 64-127)
            qT = [sb_per.tile([P, q_len], BF, name=f"qT{p}", tag=f"qT{p}")
                  for p in range(n_pairs)]
            kT = [sb_per.tile([P, k_len], BF, name=f"kT{p}", tag=f"kT{p}")
                  for p in range(n_pairs)]
            # per ktile: [V_A | 1 | pad | V_B | 1 | pad], k on partitions
            vkt = [sb_per.tile([P, n_kt * 132], BF, name=f"vkt{p}",
                               tag=f"vkt{p}") for p in range(n_pairs)]
            aT = [sb_per.tile([P, q_len], BF, name=f"aT{p}", tag=f"aT{p}")
                  for p in range(n_pairs)]

            # ---------------- phase 1: projections + V transposes, per chunk
            for p in range(n_pairs):  # ones columns (AV denominator row)
                nc.vector.memset(
                    vkt[p].rearrange("p (k a c) -> p k a c",
                                     k=n_kt, c=66)[:, :, :, 64], 1.0)
            with (
                tc.tile_pool(name="psP", bufs=2, space="PSUM") as ps_proj,
                tc.tile_pool(name="psT", bufs=2, space="PSUM") as ps_tr,
            ):
                for (s0, w) in chunks:
                    for p in range(n_pairs):
                        ps = ps_proj.tile([P, 3, 512], DT, tag="ps")
                        do_q = s0 + w > q0  # chunk overlaps the q-range
                        for dti in range(n_dt):
                            first, last = dti == 0, dti == n_dt - 1
                            if do_q:
                                nc.tensor.matmul(
                                    ps[:, 0, :w],
                                    wq_sb[:, dti, p * P:(p + 1) * P],
                                    x_sb[:, dti, s0:s0 + w],
                                    start=first, stop=last)
                            nc.tensor.matmul(
                                ps[:, 1, :w],
                                wk_sb[:, dti, p * P:(p + 1) * P],
                                x_sb[:, dti, s0:s0 + w],
                                start=first, stop=last)
                            nc.tensor.matmul(
                                ps[:, 2, :w],
                                wv_sb[:, dti, p * P:(p + 1) * P],
                                x_sb[:, dti, s0:s0 + w],
                                start=first, stop=last)
                        if do_q:  # q += bias, into persistent qT (q-range cols)
                            lo = max(s0, q0)
                            nc.vector.tensor_scalar_add(
                                qT[p][:, lo - q0:s0 + w - q0],
                                ps[:, 0, lo - s0:w], bq_sb[:, p:p + 1])
                        nc.scalar.copy(kT[p][:, s0:s0 + w], ps[:, 1, :w])
                        vt = sb_vt.tile([P, 512], BF, tag="vt")
                        nc.scalar.copy(vt[:, :w], ps[:, 2, :w])
                        for j in range(w // KT):  # transpose each ktile of V
                            k = s0 // KT + j
                            pt = ps_tr.tile([P, P], BF, tag="pt", name="pt")
                            nc.tensor.transpose(
                                pt, vt[:, j * KT:(j + 1) * KT], ident)
                            nc.scalar.copy(
                                vkt[p][:, k * 132:(k + 1) * 132].rearrange(
                                    "p (a c) -> p a c", a=2)[:, :, 0:64],
                                pt.rearrange("p (a c) -> p a c", a=2))

            # ---------------- phase 2: attention (k-outer flash, both heads
            # per k-tile; scores transposed: k on partitions, q on free axis)
            ycopy = nc.scalar.copy if cls == "early" else nc.vector.tensor_copy

            def outproj_items(si):
                """Closures: 6 mt-groups then the y DMA, for segment si."""
                h0, h1 = segs[si]
                Wd = h1 - h0
                ysb = sb_y.tile([P, n_dt, SEG], BF, tag="ysb", name="ysb")

                def mk_group(mt):
                    def go():
                        psy = ps_y.tile([P, SEG], DT, tag="psy", name="psy")
                        for p in range(n_pairs):
                            nc.tensor.matmul(
                                psy[:, :Wd], wo_sb[:, p, mt, :],
                                aT[p][:, h0:h1],
                                start=(p == 0), stop=(p == n_pairs - 1))
                        ycopy(ysb[:, mt, :Wd], psy[:, :Wd])
                    return go

                def mk_dma():
                    def go():
                        nc.sync.dma_start(
                            out=yT.rearrange("(t p) s -> p t s",
                                             p=P)[:, :, h0:h1],
                            in_=ysb[:, :, :Wd])
                    return go

                return [mk_group(mt) for mt in range(n_dt)] + [mk_dma()]

            with (
                tc.tile_pool(name="psS", bufs=2, space="PSUM") as ps_sc,
                tc.tile_pool(name="psA", bufs=3, space="PSUM") as ps_acc,
                tc.tile_pool(name="psY", bufs=1, space="PSUM") as ps_y,
            ):
                todo = []               # deferred out-proj of the previous seg
                for si, (h0, h1) in enumerate(segs):
                    Wd = h1 - h0
                    base = q0 + h0          # global q start of this segment
                    kmax = (q0 + h1 + KT - 1) // KT
                    n_iter = n_pairs * kmax
                    # deferred out-proj items go in the back half of the sweep
                    # (their aT / acc-slot deps are surely resolved by then)
                    start_it = max(2, int(n_iter * 0.45))
                    stride = max(1, (n_iter - start_it) // max(1, len(todo)))
                    emit_at = [start_it + j * stride for j in range(len(todo))]
                    it = 0
                    for p in range(n_pairs):
                        accs = [ps_acc.tile([65, SEG], DT, tag="acc",
                                            name=f"acc{hi}")
                                for hi in (0, 1)]
                        for k in range(kmax):
                            es = max(0, k * KT - base)
                            ssc = ps_sc.tile([P, 2, SEG], DT, tag="ssc")
                            for hi in (0, 1):
                                hs = slice(hi * 64, (hi + 1) * 64)
                                nc.tensor.matmul(
                                    ssc[:, hi, es:Wd],
                                    kT[p][hs, k * KT:(k + 1) * KT],
                                    qT[p][hs, h0 + es:h1],
                                    start=True, stop=True)
                            ex = sb_exp.tile([P, 2, SEG], BF, tag="ex")
                            nc.scalar.activation(
                                ex[:, :, es:Wd], ssc[:, :, es:Wd],
                                mybir.ActivationFunctionType.Exp, scale=SCALE)
                            if k * KT >= base:  # diagonal block: mask
                                de = min(es + KT, Wd)
                                for hi in (0, 1):
                                    nc.vector.tensor_mul(
                                        ex[:, hi, es:de], ex[:, hi, es:de],
                                        dmask_sb[:, :de - es])
                            for hi in (0, 1):
                                vsl = vkt[p][:, k * 132 + hi * 66:
                                             k * 132 + hi * 66 + 65]
                                nc.tensor.matmul(
                                    accs[hi][:, es:Wd], vsl, ex[:, hi, es:Wd],
                                    start=(k == 0), stop=(k == kmax - 1))
                            it += 1
                            while emit_at and it >= emit_at[0]:
                                emit_at.pop(0)
                                todo.pop(0)()
                        # normalize: a = num * (1/den), den broadcast over rows
                        for hi in (0, 1):
                            hs = slice(hi * 64, (hi + 1) * 64)
                            rr = sb_rn.tile([1, SEG], DT, tag="rr")
                            nc.vector.reciprocal(rr[:, :Wd],
                                                 accs[hi][64:65, :Wd])
                            rb = sb_rn.tile([64, SEG], DT, tag="rb")
                            nc.gpsimd.partition_broadcast(rb[:, :Wd], rr[:, :Wd])
                            nc.vector.tensor_mul(
                                aT[p][hs, h0:h1], accs[hi][0:64, :Wd],
                                rb[:, :Wd])
                    for go in todo:     # flush any leftovers
                        go()
                    todo = outproj_items(si)
                for go in todo:
                    go()

                if debug_taps:
                    for p in range(n_pairs):
                        nc.sync.dma_start(out=qTd[p], in_=qT[p])
                        nc.sync.dma_start(out=kTd[p], in_=kT[p])
                        nc.sync.dma_start(out=vktd[p], in_=vkt[p])
                        nc.sync.dma_start(out=aTd[p], in_=aT[p])

    nc.compile()
    return nc


# ---------------------------------------------------------------- host side
def _head_cols(heads):
    """column indices into a [*, 768] head-blocked axis for the given heads"""
    return np.concatenate([np.arange(h * DH, (h + 1) * DH) for h in heads])


def _bf(a):
    return np.ascontiguousarray(a.astype(ml_dtypes.bfloat16))


def make_in_maps(x, W_in, b_in, W_out):
    """Returns (late_in_maps[6], early_in_maps[2])."""
    xT = np.ascontiguousarray(x.reshape(S, D).T)          # [768, 4096]
    WT = np.ascontiguousarray(W_in.T)                     # [768, 2304]
    WoT = np.ascontiguousarray(W_out.T)                   # [768, 768]

    dm = _bf(np.triu(np.ones((P, P), np.float32)))        # k <= q (diag tile)
    xT_bf = _bf(xT)

    def core_inputs(heads, cls):
        _, q0, q1, k_len = CLASSES[cls]
        cols = _head_cols(heads)
        return {
            "xT": np.ascontiguousarray(xT_bf[:, :k_len]),
            "wqT": _bf(WT[:, cols]),
            "wkT": _bf(WT[:, 768 + cols]),
            "wvT": _bf(WT[:, 1536 + cols]),
            "bq": np.ascontiguousarray(b_in[cols][:, None]).astype(np.float32),
            "woT": _bf(WoT[cols, :]),
            "dmask": dm,
        }

    late = [core_inputs([2 * c, 2 * c + 1], "late") for c in range(6)]
    early = [core_inputs(list(range(6 * e, 6 * e + 6)), "early")
             for e in range(2)]
    return late, early


def effective_bias(b_in, W_out, b_out):
    """b_out + W_out @ b_v  (V bias folded out of the device kernel)."""
    return b_out + W_out @ b_in[1536:2304]


def assemble_output(late_res, early_res, b_eff):
    yT = np.zeros((D, S), np.float32)
    for r in late_res:
        yT[:, SPLIT:] += np.asarray(r["yT"], dtype=np.float32)
    for r in early_res:
        yT[:, :SPLIT] += np.asarray(r["yT"], dtype=np.float32)
    y = yT.T + b_eff[None, :]
    return y.reshape(B, S, D).astype(np.float32)


# ------------------------------------------- pjrt runner (explicit devices)
def _run_group(nc, in_maps, devices):
    """run_bass_via_pjrt equivalent on an explicit device subset."""
    import jax
    from jax.sharding import Mesh, PartitionSpec
    from jax.experimental.shard_map import shard_map
    from concourse import bass2jax
    from concourse.bass2jax import _bass_exec_p, partition_id_tensor

    bass2jax.install_neuronx_cc_hook()
    n_cores = len(in_maps)
    partition_name = (nc.partition_id_tensor.name
                      if nc.partition_id_tensor else None)

    in_names, out_names, out_avals, zero_outs = [], [], [], []
    for alloc in nc.m.functions[0].allocations:
        if not isinstance(alloc, mybir.MemoryLocationSet):
            continue
        name = alloc.memorylocations[0].name
        if alloc.kind == "ExternalInput":
            if name != partition_name:
                in_names.append(name)
        elif alloc.kind == "ExternalOutput":
            shape = tuple(alloc.tensor_shape)
            dtype = mybir.dt.np(alloc.dtype)
            out_names.append(name)
            out_avals.append(jax.core.ShapedArray(shape, dtype))
            zero_outs.append(np.zeros(shape, dtype))
    n_params = len(in_names)
    n_outs = len(out_avals)
    in_names = in_names + out_names
    if partition_name is not None:
        in_names.append(partition_name)
    donate = tuple(range(n_params, n_params + n_outs))

    def _body(*args):
        operands = list(args)
        if partition_name is not None:
            operands.append(partition_id_tensor())
        outs = _bass_exec_p.bind(
            *operands,
            out_avals=tuple(out_avals),
            in_names=tuple(in_names),
            out_names=tuple(out_names),
            lowering_input_output_aliases=(),
            sim_require_finite=True,
            sim_require_nnan=True,
            nc=nc,
        )
        return tuple(outs)

    per_core = [[np.asarray(m[name]) for name in in_names[:n_params]]
                for m in in_maps]
    if n_cores == 1:
        out_arrs = jax.jit(_body, donate_argnums=donate, keep_unused=True)(
            *per_core[0], *zero_outs)
        return [{n: np.asarray(out_arrs[i]) for i, n in enumerate(out_names)}]

    mesh = Mesh(np.asarray(devices), ("core",))
    in_specs = (PartitionSpec("core"),) * (n_params + n_outs)
    out_specs = (PartitionSpec("core"),) * len(out_names)
    sharded = jax.jit(
        shard_map(_body, mesh=mesh, in_specs=in_specs, out_specs=out_specs,
                  check_rep=False),
        donate_argnums=donate, keep_unused=True)
    concat_in = [np.concatenate([per_core[c][i] for c in range(n_cores)],
                                axis=0) for i in range(n_params)]
    concat_zeros = [np.zeros((n_cores * z.shape[0], *z.shape[1:]), z.dtype)
                    for z in zero_outs]
    out_arrs = sharded(*concat_in, *concat_zeros)
    return [
        {n: np.asarray(out_arrs[i]).reshape(n_cores, *out_avals[i].shape)[c]
         for i, n in enumerate(out_names)}
        for c in range(n_cores)
    ]


_MODULES = {}
_WARM = set()


def _get_module(cls):
    if cls not in _MODULES:
        _MODULES[cls] = build_module(cls)
    return _MODULES[cls]


def kernel(x, W_in, b_in, W_out, b_out):
    import jax
    x = np.asarray(x, np.float32)
    W_in = np.asarray(W_in, np.float32)
    b_in = np.asarray(b_in, np.float32)
    W_out = np.asarray(W_out, np.float32)
    b_out = np.asarray(b_out, np.float32)

    late_maps, early_maps = make_in_maps(x, W_in, b_in, W_out)
    b_eff = effective_bias(b_in, W_out, b_out)
    nc_late = _get_module("late")
    nc_early = _get_module("early")

    devs = jax.devices()
    results = {}
    errs = {}

    def run(tag, nc, maps, devices):
        try:
            results[tag] = _run_group(nc, maps, devices)
        except Exception as e:  # noqa: BLE001
            errs[tag] = e

    # first call per module compiles (serialize those); afterwards the two
    # device groups (cores 0-5 and 6-7) execute concurrently
    t1 = threading.Thread(target=run, args=("late", nc_late, late_maps, devs[0:6]))
    t2 = threading.Thread(target=run, args=("early", nc_early, early_maps, devs[6:8]))
    if not _WARM:
        t1.start(); t1.join()
        t2.start(); t2.join()
        _WARM.add(True)
    else:
        t1.start(); t2.start()
        t1.join(); t2.join()
    if errs:
        raise next(iter(errs.values()))

    return assemble_output(results["late"], results["early"], b_eff)
